# revision 25
# baseline (speedup 1.0000x reference)
"""Trainium2 Bass kernel for nn_KVCacheMemory (dual-attention memory gate).

Data-parallel over batch: each of the 8 NeuronCores computes one batch's two
single-head SxS attentions (S=4096, D=192) plus the flag-gated combine.

Per-core dataflow (all contractions ride the TensorEngine; no on-device
transposes, no vector reductions):
  - All projections run fp8 DoubleRow (contraction D=192 in one pass as
    96x2); the V projection computes both attentions' v in a single matmul
    (moving operand = [Wv_r | Wv_w], N=384).
  - scoresT[k,q] = kT.T @ qT computed directly in the transposed layout so the
    exp() output (ACT, scale=1/sqrt(D) folded in) is already the moving
    operand of the oT accumulation matmul.
  - A ones-column appended to v makes the softmax row-sum fall out of the oT
    matmul as an extra row; a unit column appended to Wo carries that row-sum
    through the output projection, so it lands as column 192 of the final
    [128,193] PSUM tile, per-partition aligned for one reciprocal + fused
    scalar_tensor_tensor (softmax normalization commutes with the linear Wo).
"""
import numpy as np
import ml_dtypes

import concourse.bacc as bacc
import concourse.tile as tile
import concourse.mybir as mybir
from concourse.bass_utils import run_bass_kernel_spmd

B, S, D = 8, 4096, 192
MEM_READ, MEM_WRITE, MEM_READY = 156, 157, 158
P = 128          # partitions / tile rows
QB = 512         # q block (matmul free dim / PSUM bank)
NQB = S // QB    # 8
KC = 128         # key chunk (contraction tile)
NKC = S // KC    # 32
NT = S // P      # 32 row tiles
D0, D1 = 128, 64  # feature split of D=192 for the oT / Wo stages
SCALE = 1.0 / float(np.sqrt(D))
F32 = mybir.dt.float32
BF16 = mybir.dt.bfloat16
FP8 = mybir.dt.float8e4
DR = mybir.MatmulPerfMode.DoubleRow
VBLK = 208       # v_ext block stride (16B-aligned for DoubleRow lhsT step)
N_CORES = 8
MULT = mybir.AluOpType.mult
ADD = mybir.AluOpType.add

_CACHE = {}


def _build():
    nc = bacc.Bacc("TRN2", target_bir_lowering=False, debug=False,
                   num_devices=N_CORES)
    x = nc.dram_tensor("x", [S, D], F32, kind="ExternalInput").ap()
    # x^T in fp8 DoubleRow layout [96, 2, S] flattened (d = 96*o + ki)
    xtd = nc.dram_tensor("xtd", [96, 2 * S], FP8, kind="ExternalInput").ap()
    # [Wq_r|Wk_r|Wq_w|Wk_w|Wv_r|Wv_w] transposed, fp8 DR layout [96, 2*6D]
    wqkvd = nc.dram_tensor("wqkvd", [96, 2 * 6 * D], FP8,
                           kind="ExternalInput").ap()
    woe0 = nc.dram_tensor("woe0", [D0, 2 * (D + 1)], BF16, kind="ExternalInput").ap()
    woe1 = nc.dram_tensor("woe1", [D1 + 1, 2 * (D + 1)], BF16, kind="ExternalInput").ap()
    params = nc.dram_tensor("params", [P, 8], F32, kind="ExternalInput").ap()
    out = nc.dram_tensor("out", [S, D], F32, kind="ExternalOutput").ap()

    with tile.TileContext(nc) as tc:
        _emit(nc, tc, x, xtd, wqkvd, woe0, woe1, params, out)
    nc.compile()
    return nc


def _emit(nc, tc, x, xtd, wqkvd, woe0, woe1, params, out):
    from contextlib import ExitStack
    with ExitStack() as st:
        cpool = st.enter_context(tc.tile_pool(name="const", bufs=1))
        bigpool = st.enter_context(tc.tile_pool(name="big", bufs=1))
        apool = st.enter_context(tc.tile_pool(name="attn", bufs=8))
        opool = st.enter_context(tc.tile_pool(name="osb", bufs=2))
        xpool = st.enter_context(tc.tile_pool(name="xin", bufs=2))
        tpool = st.enter_context(tc.tile_pool(name="tmp", bufs=3))
        # PSUM budget (8 banks): mm 3x[128,1024]=6, oT0+oT1 1x each=2;
        # res tiles rotate through the oT0 slot (tag-shared, freed post-copy)
        mmpool = st.enter_context(tc.tile_pool(name="mm", bufs=3, space="PSUM"))
        oaccpool = st.enter_context(tc.tile_pool(name="oacc", bufs=1, space="PSUM"))

        # resident constants / activations. Weights + params first (small,
        # gate everything); xtd loads chunked so phase-A chunk ci only waits
        # for its own slice.
        pp = cpool.tile([P, 8], F32, tag="pp")
        nc.sync.dma_start(pp, params)
        wqs = cpool.tile([96, 2 * 6 * D], FP8, tag="wqs")
        nc.sync.dma_start(wqs, wqkvd)
        xts = cpool.tile([96, 2 * S], FP8, tag="xts")
        for sb in range(NQB):
            for o in range(2):
                sl = slice(o * S + sb * QB, o * S + (sb + 1) * QB)
                nc.sync.dma_start(xts[:, sl], xtd[:, sl])
        woe0s = cpool.tile([D0, 2 * (D + 1)], BF16, tag="woe0s")
        nc.sync.dma_start(woe0s, woe0)
        woe1s = cpool.tile([D1 + 1, 2 * (D + 1)], BF16, tag="woe1s")
        nc.sync.dma_start(woe1s, woe1)
        # pre-fault the exp ACT table so the ~2.7us load overlaps input DMAs
        warm = cpool.tile([1, 1], F32, tag="warm")
        nc.scalar.activation(warm, pp[0:1, 0:1],
                             mybir.ActivationFunctionType.Exp)
        # HAM warm-up: ~4us of back-to-back matmuls at kernel start (during
        # the input DMA wait) so the PE clock ramps 1.2 -> 2.4 GHz before the
        # real work begins instead of ~70us in.
        wu_src = cpool.tile([P, QB], FP8, tag="wusrc")
        nc.vector.memset(wu_src, 0.0)
        # 12 back-to-back MMs span >1.5 free-running HAM windows at the cold
        # clock, so the warm transition fires before the real work begins.
        wu_ps = oaccpool.tile([P, QB], F32, tag="oT0", name="warmup")
        for i in range(12):
            nc.tensor.matmul(wu_ps, wu_src[:, 0:P], wu_src,
                             start=(i == 0), stop=(i == 11))

        wqs3 = wqs.rearrange("p (o c) -> p o c", o=2)
        xts3 = xts.rearrange("p (o s) -> p o s", o=2)

        # out accumulator [128, 32*192] f32 (tile g lives at cols g*192)
        out_acc = bigpool.tile([P, NT * D], F32, tag="out_acc")

        # per-attention persistent buffers (distinct tags so att1's phase A
        # can be emitted under att0's ACT-bound phase B)
        # v for both attentions interleaved per row tile: [t, att, VBLK] so
        # one projection matmul + ONE cast serves both (t-stride 416 = 16*26
        # keeps the DoubleRow lhsT step legal).
        vall = bigpool.tile([P, NT * 2 * VBLK], FP8, tag="vall", name="vall")
        bufs = []
        for att in range(2):
            qTd = bigpool.tile([96, 2 * S], FP8, tag=f"qTd{att}", name="qTd")
            kTd = bigpool.tile([96, 2 * S], FP8, tag=f"kTd{att}", name="kTd")
            bufs.append((qTd, kTd))

        def qk_unit(att, ci, u):
            """One fp8-DR projection matmul: u 0-1 = q halves, 2-3 = k halves
            of chunk ci for `att`. qTd/kTd layout [96, 2, S], e = 96*o + ki."""
            qTd, kTd = bufs[att]
            dst = qTd if u < 2 else kTd
            blk = 2 * att + (u // 2)       # [qr, kr, qw, kw, vr, vw] blocks
            half = u % 2
            woff = blk * D + 96 * half
            ps = mmpool.tile([P, QB], F32, tag="mm", name="ps_proj")
            nc.tensor.matmul(ps[:96, :], wqs3[:, :, woff:woff + 96],
                             xts3[:, :, ci * QB:(ci + 1) * QB],
                             start=True, stop=True, perf_mode=DR)
            nc.vector.tensor_copy(
                dst[:, half * S + ci * QB:half * S + (ci + 1) * QB],
                ps[:96, :])

        vall4 = vall.rearrange("p (t a c) -> p t a c", a=2, c=VBLK)

        def v_unit(t):
            """Row tile t of BOTH attentions' v in one fp8-DR matmul
            (moving operand = [Wv_r | Wv_w], N=384) and one strided cast."""
            if t == 0:
                nc.vector.memset(vall4[:, :, :, D:D + 1], 1.0)
            ps = mmpool.tile([P, QB], F32, tag="mm", name="ps_v")
            nc.tensor.matmul(ps[:, :2 * D], xts3[:, :, t * P:(t + 1) * P],
                             wqs3[:, :, 4 * D:6 * D],
                             start=True, stop=True, perf_mode=DR)
            nc.vector.tensor_copy(
                vall4[:, t, :, 0:D],
                ps[:, 0:2 * D].rearrange("p (a c) -> p a c", c=D))

        NPR = NKC // 2
        ostate = {}

        LAG = 3          # global oT lag (in sc-slots, carried across blocks)
        pending = []     # deferred oT emitters, one per sc-slot

        def phaseB_main(att, qb, interleave=None, pre_oT=None):
            """One q-block's score/exp stream. The oT matmuls trail LAG
            sc-slots behind in a single pipeline that runs ACROSS block
            boundaries, so score matmuls (ACT's feed) are never FIFO-blocked
            behind trailing oT work at a block switch. pre_oT (the previous
            block's epilogue) is emitted by this block's first oT pop, after
            LAG score-pairs are already queued ahead of it; the oT
            accumulators are allocated right after so the epi's res
            generations in the shared mm slot form a forward WAR chain."""
            qTd, kTd = bufs[att]
            kT3 = kTd.rearrange("p (o s) -> p o s", o=2)
            qT3 = qTd.rearrange("p (o s) -> p o s", o=2)
            ve3 = vall.rearrange("p (t c) -> p t c", c=2 * VBLK)[
                :, :, att * VBLK:(att + 1) * VBLK]
            qs3 = qT3[:, :, qb * QB:(qb + 1) * QB]

            def make_ot(pr, at3):
                def emit():
                    if pr == 0:
                        if pre_oT is not None:
                            pre_oT()
                        ostate[(att, qb)] = (
                            oaccpool.tile([P, QB], F32, tag="oT0",
                                          name="oT0"),
                            oaccpool.tile([D1 + 1, QB], F32, tag="oT1",
                                          name="oT1"))
                    oT0, oT1 = ostate[(att, qb)]
                    nc.tensor.matmul(oT0, ve3[:, 2 * pr:2 * pr + 2, 0:D0],
                                     at3, start=(pr == 0),
                                     stop=(pr == NPR - 1), perf_mode=DR)
                    nc.tensor.matmul(oT1, ve3[:, 2 * pr:2 * pr + 2, D0:D + 1],
                                     at3, start=(pr == 0),
                                     stop=(pr == NPR - 1), perf_mode=DR)
                return emit

            for pr in range(NPR):
                # two key-chunks' scoresT side by side in one 2-bank tile
                sc = mmpool.tile([P, 2 * QB], F32, tag="mm", name="sc")
                for h in range(2):
                    kc = 2 * pr + h
                    nc.tensor.matmul(sc[:, h * QB:(h + 1) * QB],
                                     kT3[:, :, kc * KC:(kc + 1) * KC],
                                     qs3, start=True, stop=True,
                                     perf_mode=DR)
                at = apool.tile([P, 2 * QB], FP8, tag="at")
                nc.scalar.activation(at, sc, mybir.ActivationFunctionType.Exp,
                                     scale=SCALE)
                pending.append(make_ot(pr, at.rearrange("p (o n) -> p o n",
                                                        o=2)))
                if len(pending) > LAG:
                    pending.pop(0)()
                if interleave is not None:
                    interleave(pr)

        def phaseB_epi(att, qb, final=False):
            wo_off = att * (D + 1)
            fc = 1 + att
            oT0, oT1 = ostate.pop((att, qb))
            oT0s = opool.tile([P, QB], BF16, tag="oT0s")
            nc.vector.tensor_copy(oT0s, oT0)
            oT1s = opool.tile([D1 + 1, QB], BF16, tag="oT1s")
            nc.vector.tensor_copy(oT1s, oT1)

            if att == 0:
                xt4 = xpool.tile([P, 4 * D], F32, tag="xt")
                for qt in range(4):
                    g = qb * 4 + qt
                    nc.sync.dma_start(xt4[:, qt * D:(qt + 1) * D],
                                      x[g * P:(g + 1) * P, :])
            for qt in range(4):
                g = qb * 4 + qt
                # res rides the mm slot; the quick resS copy frees it so the
                # next block's score matmuls rotate through unimpeded while
                # the normalize chain reads the SBUF copy off-path.
                res = mmpool.tile([P, QB], F32, tag="mm", name="res")
                res = res[:, 0:D + 1]
                nc.tensor.matmul(res, oT0s[:, qt * P:(qt + 1) * P],
                                 woe0s[:, wo_off:wo_off + D + 1],
                                 start=True, stop=False)
                nc.tensor.matmul(res, oT1s[:, qt * P:(qt + 1) * P],
                                 woe1s[:, wo_off:wo_off + D + 1],
                                 start=False, stop=True)
                resS = tpool.tile([P, D + 1], F32, tag="resS")
                nc.vector.tensor_copy(resS, res)
                rec = tpool.tile([P, 1], F32, tag="rec")
                nc.vector.reciprocal(rec, resS[:, D:D + 1])
                recf = tpool.tile([P, 1], F32, tag="recf")
                nc.vector.tensor_scalar(recf, rec, pp[:, fc:fc + 1], None,
                                        op0=MULT)
                acc = out_acc[:, g * D:(g + 1) * D]
                if att == 0:
                    nc.vector.tensor_scalar(acc, xt4[:, qt * D:(qt + 1) * D],
                                            pp[:, 0:1], None, op0=MULT)
                nc.vector.scalar_tensor_tensor(acc, resS[:, 0:D], recf, acc,
                                               op0=MULT, op1=ADD)
                if att == 1 and final:
                    # last block: finish + ship each group as soon as its
                    # normalize lands so the out DMAs overlap the chain
                    nc.vector.memset(acc[:, MEM_READ:MEM_WRITE + 1], 0.0)
                    nc.vector.tensor_copy(acc[:, MEM_READY:MEM_READY + 1],
                                          pp[:, 3:4])
                    nc.sync.dma_start(out[g * P:(g + 1) * P, :], acc)
            if att == 1 and not final:
                a4 = out_acc.rearrange("p (t c) -> p t c", c=D)[
                    :, qb * 4:(qb + 1) * 4, :]
                nc.vector.memset(a4[:, :, MEM_READ:MEM_WRITE + 1], 0.0)
                nc.vector.tensor_copy(a4[:, :, MEM_READY:MEM_READY + 1],
                                      pp[:, 4:8])
                for qt in range(4):
                    g = qb * 4 + qt
                    nc.sync.dma_start(out[g * P:(g + 1) * P, :],
                                      out_acc[:, g * D:(g + 1) * D])

        # driver: A(0,0)+v(0) head feeds B(0,0); remaining k/v stream JIT
        # under B(0,0) (chunk ci ready one pr-pair before its first use);
        # att1 q/k and att0's next q ride under B(0,qb); epilogues deferred
        # one qb so the next qb's score matmuls keep ACT fed.
        KQ = (2, 3, 0, 1)   # k halves first, then q halves

        def ilv0(pr):
            # JIT prep under B(0,0): k chunk pr//2+1 feeds sc_act(2c) at slot
            # 2c; v chunk pr//2 feeds ot(2c) emitted at slot 2c+HEAD (looser
            # deadline thanks to the oT lag, so v chunk 0 rides slots 0-1
            # instead of the pre-loop head).
            ck = pr // 2 + 1
            cv = pr // 2
            if pr % 2 == 0:
                if ck < NQB:
                    qk_unit(0, ck, 2)
                v_unit(4 * cv + 0)
                v_unit(4 * cv + 1)
            else:
                if ck < NQB:
                    qk_unit(0, ck, 3)
                v_unit(4 * cv + 2)
                v_unit(4 * cv + 3)
            if pr in (14, 15):
                qk_unit(0, 1, pr - 14)

        # A-units ride late prs so their DVE casts queue after the epilogue's
        # DVE chain (which runs at the head of each block).
        def ilv_b0(qb):
            def f(pr):
                if pr >= 8 and pr % 2 == 0:
                    qk_unit(1, qb - 1, KQ[(pr - 8) // 2])
                elif pr in (9, 11) and qb + 1 < NQB:
                    qk_unit(0, qb + 1, (pr - 9) // 2)
            return f

        def ilv_a1_last(pr):
            if pr >= 8 and pr % 2 == 0:
                qk_unit(1, NQB - 1, KQ[(pr - 8) // 2])

        def epi_hook(att, qb):
            return lambda: phaseB_epi(att, qb)

        for u in KQ:
            qk_unit(0, 0, u)
        for j in range(4):
            v_unit(j)
        phaseB_main(0, 0, interleave=ilv0)
        for qb in range(1, NQB):
            phaseB_main(0, qb, interleave=ilv_b0(qb),
                        pre_oT=epi_hook(0, qb - 1))
        # A(1) chunk 7 rides under B(1,0)'s first pairs
        phaseB_main(1, 0, interleave=ilv_a1_last,
                    pre_oT=epi_hook(0, NQB - 1))
        for qb in range(1, NQB):
            phaseB_main(1, qb, pre_oT=epi_hook(1, qb - 1))
        while pending:
            pending.pop(0)()
        phaseB_epi(1, NQB - 1, final=True)


def _prep_core_inputs(x_full, weights):
    """Host-side shard/layout prep. weights: dict of the 8 [192,192] f32."""
    bf = ml_dtypes.bfloat16
    f8 = ml_dtypes.float8_e4m3

    def to_dr(a):  # [192, C] -> DoubleRow layout [96, 2*C], d = 96*o + ki
        c = a.shape[1]
        return np.ascontiguousarray(
            a.reshape(2, 96, c).transpose(1, 0, 2).reshape(96, 2 * c))

    worder = ["Wq_r", "Wk_r", "Wq_w", "Wk_w", "Wv_r", "Wv_w"]
    wcat = np.concatenate([np.ascontiguousarray(weights[n].T) for n in worder],
                          axis=1)  # [192, 6*192]
    wqkvd = to_dr(wcat).astype(f8)
    woe = np.zeros((D + 1, 2 * (D + 1)), np.float32)
    for a, n in enumerate(("Wo_r", "Wo_w")):
        woe[:D, a * (D + 1):a * (D + 1) + D] = weights[n].T
        woe[D, a * (D + 1) + D] = 1.0
    woe = woe.astype(bf)
    in_maps = []
    for c in range(N_CORES):
        xb = np.ascontiguousarray(x_full[c]).astype(np.float32)  # [4096,192]
        xT = np.ascontiguousarray(xb.T)                          # [192,4096]
        rg = float(xb[0, MEM_READ])
        wg = float(xb[0, MEM_WRITE])
        ready = rg + wg
        pvec = np.array([1.0 - rg - wg, rg, wg, ready,
                         ready, ready, ready, ready], np.float32)
        in_maps.append({
            "x": xb,
            "xtd": to_dr(xT).astype(f8),
            "wqkvd": wqkvd,
            "woe0": np.ascontiguousarray(woe[:D0]),
            "woe1": np.ascontiguousarray(woe[D0:]),
            "params": np.tile(pvec, (P, 1)),
        })
    return in_maps


def _run(inputs, **spmd_kwargs):
    if "nc" not in _CACHE:
        _CACHE["nc"] = _build()
    nc = _CACHE["nc"]
    x_full = np.asarray(inputs["x"], np.float32)
    weights = {k: np.asarray(inputs[k], np.float32) for k in
               ("Wq_r", "Wk_r", "Wv_r", "Wo_r", "Wq_w", "Wk_w", "Wv_w", "Wo_w")}
    in_maps = _prep_core_inputs(x_full, weights)
    res = run_bass_kernel_spmd(nc, in_maps, list(range(N_CORES)), **spmd_kwargs)
    out = np.stack([res.results[c]["out"] for c in range(N_CORES)], axis=0)
    return out.astype(np.float32), res


def kernel(**inputs):
    out, _ = _run(inputs)
    return out


def kernel_traced(**inputs):
    """For test.py: also returns BassKernelResults with profile info."""
    return _run(inputs, trace=True)


# revision 27
# speedup vs baseline: 1.0203x; 1.0203x over previous
"""Trainium2 Bass kernel for nn_KVCacheMemory (dual-attention memory gate).

Data-parallel over batch: each of the 8 NeuronCores computes one batch's two
single-head SxS attentions (S=4096, D=192) plus the flag-gated combine.

Per-core dataflow (all contractions ride the TensorEngine; no on-device
transposes, no vector reductions):
  - All projections run fp8 DoubleRow (contraction D=192 in one pass as
    96x2); the V projection computes both attentions' v in a single matmul
    (moving operand = [Wv_r | Wv_w], N=384).
  - scoresT[k,q] = kT.T @ qT computed directly in the transposed layout so the
    exp() output (ACT, scale=1/sqrt(D) folded in) is already the moving
    operand of the oT accumulation matmul.
  - A ones-column appended to v makes the softmax row-sum fall out of the oT
    matmul as an extra row; a unit column appended to Wo carries that row-sum
    through the output projection, so it lands as column 192 of the final
    [128,193] PSUM tile, per-partition aligned for one reciprocal + fused
    scalar_tensor_tensor (softmax normalization commutes with the linear Wo).
"""
import numpy as np
import ml_dtypes

import concourse.bacc as bacc
import concourse.tile as tile
import concourse.mybir as mybir
from concourse.bass_utils import run_bass_kernel_spmd

B, S, D = 8, 4096, 192
MEM_READ, MEM_WRITE, MEM_READY = 156, 157, 158
P = 128          # partitions / tile rows
QB = 512         # q block (matmul free dim / PSUM bank)
NQB = S // QB    # 8
KC = 128         # key chunk (contraction tile)
NKC = S // KC    # 32
NT = S // P      # 32 row tiles
D0, D1 = 128, 64  # feature split of D=192 for the oT / Wo stages
SCALE = 1.0 / float(np.sqrt(D))
F32 = mybir.dt.float32
BF16 = mybir.dt.bfloat16
FP8 = mybir.dt.float8e4
DR = mybir.MatmulPerfMode.DoubleRow
VBLK = 208       # v_ext block stride (16B-aligned for DoubleRow lhsT step)
N_CORES = 8
MULT = mybir.AluOpType.mult
ADD = mybir.AluOpType.add

_CACHE = {}


def _build():
    nc = bacc.Bacc("TRN2", target_bir_lowering=False, debug=False,
                   num_devices=N_CORES)
    x = nc.dram_tensor("x", [S, D], F32, kind="ExternalInput").ap()
    # x^T in fp8 DoubleRow layout [96, 2, S] flattened (d = 96*o + ki)
    xtd = nc.dram_tensor("xtd", [96, 2 * S], FP8, kind="ExternalInput").ap()
    # [Wq_r|Wk_r|Wq_w|Wk_w|Wv_r|Wv_w] transposed, fp8 DR layout [96, 2*6D]
    wqkvd = nc.dram_tensor("wqkvd", [96, 2 * 6 * D], FP8,
                           kind="ExternalInput").ap()
    woe0 = nc.dram_tensor("woe0", [D0, 2 * (D + 1)], BF16, kind="ExternalInput").ap()
    woe1 = nc.dram_tensor("woe1", [D1 + 1, 2 * (D + 1)], BF16, kind="ExternalInput").ap()
    params = nc.dram_tensor("params", [P, 8], F32, kind="ExternalInput").ap()
    out = nc.dram_tensor("out", [S, D], F32, kind="ExternalOutput").ap()

    with tile.TileContext(nc) as tc:
        _emit(nc, tc, x, xtd, wqkvd, woe0, woe1, params, out)
    nc.compile()
    return nc


def _emit(nc, tc, x, xtd, wqkvd, woe0, woe1, params, out):
    from contextlib import ExitStack
    with ExitStack() as st:
        cpool = st.enter_context(tc.tile_pool(name="const", bufs=1))
        bigpool = st.enter_context(tc.tile_pool(name="big", bufs=1))
        apool = st.enter_context(tc.tile_pool(name="attn", bufs=8))
        opool = st.enter_context(tc.tile_pool(name="osb", bufs=2))
        xpool = st.enter_context(tc.tile_pool(name="xin", bufs=2))
        tpool = st.enter_context(tc.tile_pool(name="tmp", bufs=3))
        # PSUM budget (8 banks): mm 3x[128,1024]=6, oT0+oT1 1x each=2;
        # res tiles rotate through the oT0 slot (tag-shared, freed post-copy)
        mmpool = st.enter_context(tc.tile_pool(name="mm", bufs=3, space="PSUM"))
        oaccpool = st.enter_context(tc.tile_pool(name="oacc", bufs=1, space="PSUM"))

        # resident constants / activations. Weights + params first (small,
        # gate everything); xtd loads chunked so phase-A chunk ci only waits
        # for its own slice.
        pp = cpool.tile([P, 8], F32, tag="pp")
        nc.sync.dma_start(pp, params)
        wqs = cpool.tile([96, 2 * 6 * D], FP8, tag="wqs")
        nc.sync.dma_start(wqs, wqkvd)
        xts = cpool.tile([96, 2 * S], FP8, tag="xts")
        for sb in range(NQB):
            for o in range(2):
                sl = slice(o * S + sb * QB, o * S + (sb + 1) * QB)
                nc.sync.dma_start(xts[:, sl], xtd[:, sl])
        woe0s = cpool.tile([D0, 2 * (D + 1)], BF16, tag="woe0s")
        nc.sync.dma_start(woe0s, woe0)
        woe1s = cpool.tile([D1 + 1, 2 * (D + 1)], BF16, tag="woe1s")
        nc.sync.dma_start(woe1s, woe1)
        # pre-fault the exp ACT table so the ~2.7us load overlaps input DMAs
        warm = cpool.tile([1, 1], F32, tag="warm")
        nc.scalar.activation(warm, pp[0:1, 0:1],
                             mybir.ActivationFunctionType.Exp)
        # HAM warm-up: ~4us of back-to-back matmuls at kernel start (during
        # the input DMA wait) so the PE clock ramps 1.2 -> 2.4 GHz before the
        # real work begins instead of ~70us in.
        wu_src = cpool.tile([P, QB], FP8, tag="wusrc")
        nc.vector.memset(wu_src, 0.0)
        # 12 back-to-back MMs span >1.5 free-running HAM windows at the cold
        # clock, so the warm transition fires before the real work begins.
        wu_ps = oaccpool.tile([P, QB], F32, tag="oT0", name="warmup")
        for i in range(12):
            nc.tensor.matmul(wu_ps, wu_src[:, 0:P], wu_src,
                             start=(i == 0), stop=(i == 11))

        wqs3 = wqs.rearrange("p (o c) -> p o c", o=2)
        xts3 = xts.rearrange("p (o s) -> p o s", o=2)

        # out accumulator [128, 32*192] f32 (tile g lives at cols g*192)
        out_acc = bigpool.tile([P, NT * D], F32, tag="out_acc")

        # per-attention persistent buffers (distinct tags so att1's phase A
        # can be emitted under att0's ACT-bound phase B)
        # v for both attentions interleaved per row tile: [t, att, VBLK] so
        # one projection matmul + ONE cast serves both (t-stride 416 = 16*26
        # keeps the DoubleRow lhsT step legal).
        vall = bigpool.tile([P, NT * 2 * VBLK], FP8, tag="vall", name="vall")
        bufs = []
        for att in range(2):
            qTd = bigpool.tile([96, 2 * S], FP8, tag=f"qTd{att}", name="qTd")
            kTd = bigpool.tile([96, 2 * S], FP8, tag=f"kTd{att}", name="kTd")
            bufs.append((qTd, kTd))

        def qk_unit(att, ci, u):
            """One fp8-DR projection matmul: u 0-1 = q halves, 2-3 = k halves
            of chunk ci for `att`. qTd/kTd layout [96, 2, S], e = 96*o + ki."""
            qTd, kTd = bufs[att]
            dst = qTd if u < 2 else kTd
            blk = 2 * att + (u // 2)       # [qr, kr, qw, kw, vr, vw] blocks
            half = u % 2
            woff = blk * D + 96 * half
            ps = mmpool.tile([P, QB], F32, tag="mm", name="ps_proj")
            nc.tensor.matmul(ps[:96, :], wqs3[:, :, woff:woff + 96],
                             xts3[:, :, ci * QB:(ci + 1) * QB],
                             start=True, stop=True, perf_mode=DR)
            nc.vector.tensor_copy(
                dst[:, half * S + ci * QB:half * S + (ci + 1) * QB],
                ps[:96, :])

        vall4 = vall.rearrange("p (t a c) -> p t a c", a=2, c=VBLK)

        def v_unit(t):
            """Row tile t of BOTH attentions' v in one fp8-DR matmul
            (moving operand = [Wv_r | Wv_w], N=384) and one strided cast."""
            if t == 0:
                nc.vector.memset(vall4[:, :, :, D:D + 1], 1.0)
            ps = mmpool.tile([P, QB], F32, tag="mm", name="ps_v")
            nc.tensor.matmul(ps[:, :2 * D], xts3[:, :, t * P:(t + 1) * P],
                             wqs3[:, :, 4 * D:6 * D],
                             start=True, stop=True, perf_mode=DR)
            nc.vector.tensor_copy(
                vall4[:, t, :, 0:D],
                ps[:, 0:2 * D].rearrange("p (a c) -> p a c", c=D))

        NPR = NKC // 2
        ostate = {}

        def phaseB_main(att, qb, interleave=None, pre_oT=None):
            """The first HEAD score-pairs + ACTs are emitted before pre_oT
            (the deferred previous epilogue) so the PE has dependency-ready
            work queued ahead of the epilogue matmuls (which wait on DVE
            copies), and each ot(pr-HEAD) rides with sc_act(pr) so score
            matmuls (ACT's feed) are never FIFO-blocked behind oT work. The
            oT accumulators are allocated after pre_oT so the epi's res
            generations in the shared mm slot form a forward WAR chain."""
            qTd, kTd = bufs[att]
            kT3 = kTd.rearrange("p (o s) -> p o s", o=2)
            qT3 = qTd.rearrange("p (o s) -> p o s", o=2)
            ve3 = vall.rearrange("p (t c) -> p t c", c=2 * VBLK)[
                :, :, att * VBLK:(att + 1) * VBLK]
            qs3 = qT3[:, :, qb * QB:(qb + 1) * QB]
            HEAD = 3
            at3s = {}

            def sc_act(pr):
                # two key-chunks' scoresT side by side in one 2-bank tile
                sc = mmpool.tile([P, 2 * QB], F32, tag="mm", name="sc")
                for h in range(2):
                    kc = 2 * pr + h
                    nc.tensor.matmul(sc[:, h * QB:(h + 1) * QB],
                                     kT3[:, :, kc * KC:(kc + 1) * KC],
                                     qs3, start=True, stop=True,
                                     perf_mode=DR)
                at = apool.tile([P, 2 * QB], FP8, tag="at")
                nc.scalar.activation(at, sc, mybir.ActivationFunctionType.Exp,
                                     scale=SCALE)
                at3s[pr] = at.rearrange("p (o n) -> p o n", o=2)

            def ot(pr, oT0, oT1):
                at3 = at3s.pop(pr)
                nc.tensor.matmul(oT0, ve3[:, 2 * pr:2 * pr + 2, 0:D0], at3,
                                 start=(pr == 0), stop=(pr == NPR - 1),
                                 perf_mode=DR)
                nc.tensor.matmul(oT1, ve3[:, 2 * pr:2 * pr + 2, D0:D + 1],
                                 at3, start=(pr == 0), stop=(pr == NPR - 1),
                                 perf_mode=DR)

            for pr in range(HEAD):
                sc_act(pr)
                if interleave is not None:
                    interleave(pr)
            if pre_oT is not None:
                pre_oT()
            oT0 = oaccpool.tile([P, QB], F32, tag="oT0")
            oT1 = oaccpool.tile([D1 + 1, QB], F32, tag="oT1")
            ostate[(att, qb)] = (oT0, oT1)
            for pr in range(HEAD, NPR):
                sc_act(pr)
                ot(pr - HEAD, oT0, oT1)
                if interleave is not None:
                    interleave(pr)
            for pr in range(NPR - HEAD, NPR):
                ot(pr, oT0, oT1)

        def phaseB_epi(att, qb, final=False):
            wo_off = att * (D + 1)
            fc = 1 + att
            oT0, oT1 = ostate.pop((att, qb))
            oT0s = opool.tile([P, QB], BF16, tag="oT0s")
            nc.vector.tensor_copy(oT0s, oT0)
            oT1s = opool.tile([D1 + 1, QB], BF16, tag="oT1s")
            nc.vector.tensor_copy(oT1s, oT1)

            if att == 0:
                xt4 = xpool.tile([P, 4 * D], F32, tag="xt")
                for qt in range(4):
                    g = qb * 4 + qt
                    nc.sync.dma_start(xt4[:, qt * D:(qt + 1) * D],
                                      x[g * P:(g + 1) * P, :])
            for qt in range(4):
                g = qb * 4 + qt
                # res rides the mm slot; the quick resS copy frees it so the
                # next block's score matmuls rotate through unimpeded while
                # the normalize chain reads the SBUF copy off-path.
                res = mmpool.tile([P, QB], F32, tag="mm", name="res")
                res = res[:, 0:D + 1]
                nc.tensor.matmul(res, oT0s[:, qt * P:(qt + 1) * P],
                                 woe0s[:, wo_off:wo_off + D + 1],
                                 start=True, stop=False)
                nc.tensor.matmul(res, oT1s[:, qt * P:(qt + 1) * P],
                                 woe1s[:, wo_off:wo_off + D + 1],
                                 start=False, stop=True)
                resS = tpool.tile([P, D + 1], F32, tag="resS")
                nc.vector.tensor_copy(resS, res)
                rec = tpool.tile([P, 1], F32, tag="rec")
                nc.vector.reciprocal(rec, resS[:, D:D + 1])
                recf = tpool.tile([P, 1], F32, tag="recf")
                nc.vector.tensor_scalar(recf, rec, pp[:, fc:fc + 1], None,
                                        op0=MULT)
                acc = out_acc[:, g * D:(g + 1) * D]
                if att == 0:
                    nc.vector.tensor_scalar(acc, xt4[:, qt * D:(qt + 1) * D],
                                            pp[:, 0:1], None, op0=MULT)
                nc.vector.scalar_tensor_tensor(acc, resS[:, 0:D], recf, acc,
                                               op0=MULT, op1=ADD)
                if att == 1 and final:
                    # last block: finish + ship each group as soon as its
                    # normalize lands so the out DMAs overlap the chain
                    nc.vector.memset(acc[:, MEM_READ:MEM_WRITE + 1], 0.0)
                    nc.vector.tensor_copy(acc[:, MEM_READY:MEM_READY + 1],
                                          pp[:, 3:4])
                    nc.sync.dma_start(out[g * P:(g + 1) * P, :], acc)
            if att == 1 and not final:
                a4 = out_acc.rearrange("p (t c) -> p t c", c=D)[
                    :, qb * 4:(qb + 1) * 4, :]
                nc.vector.memset(a4[:, :, MEM_READ:MEM_WRITE + 1], 0.0)
                nc.vector.tensor_copy(a4[:, :, MEM_READY:MEM_READY + 1],
                                      pp[:, 4:8])
                for qt in range(4):
                    g = qb * 4 + qt
                    nc.sync.dma_start(out[g * P:(g + 1) * P, :],
                                      out_acc[:, g * D:(g + 1) * D])

        # driver: A(0,0)+v(0) head feeds B(0,0); remaining k/v stream JIT
        # under B(0,0) (chunk ci ready one pr-pair before its first use);
        # att1 q/k and att0's next q ride under B(0,qb); epilogues deferred
        # one qb so the next qb's score matmuls keep ACT fed.
        KQ = (2, 3, 0, 1)   # k halves first, then q halves

        def ilv0(pr):
            # JIT prep under B(0,0): k chunk pr//2+1 feeds sc_act(2c) at slot
            # 2c; v chunk pr//2 feeds ot(2c) emitted at slot 2c+HEAD (looser
            # deadline thanks to the oT lag, so v chunk 0 rides slots 0-1
            # instead of the pre-loop head).
            ck = pr // 2 + 1
            cv = pr // 2
            if pr % 2 == 0:
                if ck < NQB:
                    qk_unit(0, ck, 2)
                v_unit(4 * cv + 0)
                v_unit(4 * cv + 1)
            else:
                if ck < NQB:
                    qk_unit(0, ck, 3)
                v_unit(4 * cv + 2)
                v_unit(4 * cv + 3)
            if pr in (14, 15):
                qk_unit(0, 1, pr - 14)

        # A-units ride late prs so their DVE casts queue after the epilogue's
        # DVE chain (which runs at the head of each block).
        def ilv_b0(qb):
            def f(pr):
                if pr >= 8 and pr % 2 == 0:
                    qk_unit(1, qb - 1, KQ[(pr - 8) // 2])
                elif pr in (9, 11) and qb + 1 < NQB:
                    qk_unit(0, qb + 1, (pr - 9) // 2)
            return f

        def ilv_a1_last(pr):
            if pr >= 8 and pr % 2 == 0:
                qk_unit(1, NQB - 1, KQ[(pr - 8) // 2])

        def epi_hook(att, qb):
            return lambda: phaseB_epi(att, qb)

        for u in KQ:
            qk_unit(0, 0, u)
        for j in range(4):
            v_unit(j)
        phaseB_main(0, 0, interleave=ilv0)
        for qb in range(1, NQB):
            phaseB_main(0, qb, interleave=ilv_b0(qb),
                        pre_oT=epi_hook(0, qb - 1))
        # A(1) chunk 7 rides under B(1,0)'s first pairs
        phaseB_main(1, 0, interleave=ilv_a1_last,
                    pre_oT=epi_hook(0, NQB - 1))
        for qb in range(1, NQB):
            phaseB_main(1, qb, pre_oT=epi_hook(1, qb - 1))
        phaseB_epi(1, NQB - 1, final=True)


def _prep_core_inputs(x_full, weights):
    """Host-side shard/layout prep. weights: dict of the 8 [192,192] f32."""
    bf = ml_dtypes.bfloat16
    f8 = ml_dtypes.float8_e4m3

    def to_dr(a):  # [192, C] -> DoubleRow layout [96, 2*C], d = 96*o + ki
        c = a.shape[1]
        return np.ascontiguousarray(
            a.reshape(2, 96, c).transpose(1, 0, 2).reshape(96, 2 * c))

    worder = ["Wq_r", "Wk_r", "Wq_w", "Wk_w", "Wv_r", "Wv_w"]
    wcat = np.concatenate([np.ascontiguousarray(weights[n].T) for n in worder],
                          axis=1)  # [192, 6*192]
    wqkvd = to_dr(wcat).astype(f8)
    woe = np.zeros((D + 1, 2 * (D + 1)), np.float32)
    for a, n in enumerate(("Wo_r", "Wo_w")):
        woe[:D, a * (D + 1):a * (D + 1) + D] = weights[n].T
        woe[D, a * (D + 1) + D] = 1.0
    woe = woe.astype(bf)
    in_maps = []
    for c in range(N_CORES):
        xb = np.ascontiguousarray(x_full[c]).astype(np.float32)  # [4096,192]
        xT = np.ascontiguousarray(xb.T)                          # [192,4096]
        rg = float(xb[0, MEM_READ])
        wg = float(xb[0, MEM_WRITE])
        ready = rg + wg
        pvec = np.array([1.0 - rg - wg, rg, wg, ready,
                         ready, ready, ready, ready], np.float32)
        in_maps.append({
            "x": xb,
            "xtd": to_dr(xT).astype(f8),
            "wqkvd": wqkvd,
            "woe0": np.ascontiguousarray(woe[:D0]),
            "woe1": np.ascontiguousarray(woe[D0:]),
            "params": np.tile(pvec, (P, 1)),
        })
    return in_maps


def _run(inputs, **spmd_kwargs):
    if "nc" not in _CACHE:
        _CACHE["nc"] = _build()
    nc = _CACHE["nc"]
    x_full = np.asarray(inputs["x"], np.float32)
    weights = {k: np.asarray(inputs[k], np.float32) for k in
               ("Wq_r", "Wk_r", "Wv_r", "Wo_r", "Wq_w", "Wk_w", "Wv_w", "Wo_w")}
    in_maps = _prep_core_inputs(x_full, weights)
    res = run_bass_kernel_spmd(nc, in_maps, list(range(N_CORES)), **spmd_kwargs)
    out = np.stack([res.results[c]["out"] for c in range(N_CORES)], axis=0)
    return out.astype(np.float32), res


def kernel(**inputs):
    out, _ = _run(inputs)
    return out


def kernel_traced(**inputs):
    """For test.py: also returns BassKernelResults with profile info."""
    return _run(inputs, trace=True)


# revision 32
# speedup vs baseline: 1.0350x; 1.0144x over previous
"""Trainium2 Bass kernel for nn_KVCacheMemory (dual-attention memory gate).

Data-parallel over batch: each of the 8 NeuronCores computes one batch's two
single-head SxS attentions (S=4096, D=192) plus the flag-gated combine.

Per-core dataflow (all contractions ride the TensorEngine; no on-device
transposes, no vector reductions):
  - All projections run fp8 DoubleRow (contraction D=192 in one pass as
    96x2); the V projection computes both attentions' v in a single matmul
    (moving operand = [Wv_r | Wv_w], N=384).
  - scoresT[k,q] = kT.T @ qT computed directly in the transposed layout so the
    exp() output (ACT, scale=1/sqrt(D) folded in) is already the moving
    operand of the oT accumulation matmul.
  - A ones-column appended to v makes the softmax row-sum fall out of the oT
    matmul as an extra row; a unit column appended to Wo carries that row-sum
    through the output projection, so it lands as column 192 of the final
    [128,193] PSUM tile, per-partition aligned for one reciprocal + fused
    scalar_tensor_tensor (softmax normalization commutes with the linear Wo).
"""
import numpy as np
import ml_dtypes

import concourse.bacc as bacc
import concourse.tile as tile
import concourse.mybir as mybir
from concourse.bass_utils import run_bass_kernel_spmd

B, S, D = 8, 4096, 192
MEM_READ, MEM_WRITE, MEM_READY = 156, 157, 158
P = 128          # partitions / tile rows
QB = 512         # q block (matmul free dim / PSUM bank)
NQB = S // QB    # 8
KC = 128         # key chunk (contraction tile)
NKC = S // KC    # 32
NT = S // P      # 32 row tiles
D0, D1 = 128, 64  # feature split of D=192 for the oT / Wo stages
SCALE = 1.0 / float(np.sqrt(D))
F32 = mybir.dt.float32
BF16 = mybir.dt.bfloat16
FP8 = mybir.dt.float8e4
DR = mybir.MatmulPerfMode.DoubleRow
VBLK = 208       # v_ext block stride (16B-aligned for DoubleRow lhsT step)
N_CORES = 8
MULT = mybir.AluOpType.mult
ADD = mybir.AluOpType.add

_CACHE = {}


def _build():
    nc = bacc.Bacc("TRN2", target_bir_lowering=False, debug=False,
                   num_devices=N_CORES)
    x = nc.dram_tensor("x", [S, D], F32, kind="ExternalInput").ap()
    # x^T in fp8 DoubleRow layout [96, 2, S] flattened (d = 96*o + ki)
    xtd = nc.dram_tensor("xtd", [96, 2 * S], FP8, kind="ExternalInput").ap()
    # [Wq_r|Wk_r|Wq_w|Wk_w|Wv_r|Wv_w] transposed, fp8 DR layout [96, 2*6D]
    wqkvd = nc.dram_tensor("wqkvd", [96, 2 * 6 * D], FP8,
                           kind="ExternalInput").ap()
    woe0 = nc.dram_tensor("woe0", [D0, 2 * (D + 1)], BF16, kind="ExternalInput").ap()
    woe1 = nc.dram_tensor("woe1", [D1 + 1, 2 * (D + 1)], BF16, kind="ExternalInput").ap()
    params = nc.dram_tensor("params", [P, 8], F32, kind="ExternalInput").ap()
    out = nc.dram_tensor("out", [S, D], F32, kind="ExternalOutput").ap()

    with tile.TileContext(nc) as tc:
        _emit(nc, tc, x, xtd, wqkvd, woe0, woe1, params, out)
    nc.compile()
    return nc


def _emit(nc, tc, x, xtd, wqkvd, woe0, woe1, params, out):
    from contextlib import ExitStack
    with ExitStack() as st:
        cpool = st.enter_context(tc.tile_pool(name="const", bufs=1))
        bigpool = st.enter_context(tc.tile_pool(name="big", bufs=1))
        apool = st.enter_context(tc.tile_pool(name="attn", bufs=8))
        opool = st.enter_context(tc.tile_pool(name="osb", bufs=2))
        xpool = st.enter_context(tc.tile_pool(name="xin", bufs=2))
        tpool = st.enter_context(tc.tile_pool(name="tmp", bufs=3))
        # PSUM budget (8 banks): mm 3x[128,1024]=6, oT0+oT1 1x each=2;
        # res tiles rotate through the oT0 slot (tag-shared, freed post-copy)
        mmpool = st.enter_context(tc.tile_pool(name="mm", bufs=3, space="PSUM"))
        oaccpool = st.enter_context(tc.tile_pool(name="oacc", bufs=1, space="PSUM"))

        # resident constants / activations. Weights + params first (small,
        # gate everything); xtd loads chunked so phase-A chunk ci only waits
        # for its own slice.
        pp = cpool.tile([P, 8], F32, tag="pp")
        nc.sync.dma_start(pp, params)
        wqs = cpool.tile([96, 2 * 6 * D], FP8, tag="wqs")
        nc.sync.dma_start(wqs, wqkvd)
        xts = cpool.tile([96, 2 * S], FP8, tag="xts")
        for sb in range(NQB):
            for o in range(2):
                sl = slice(o * S + sb * QB, o * S + (sb + 1) * QB)
                nc.sync.dma_start(xts[:, sl], xtd[:, sl])
        woe0s = cpool.tile([D0, 2 * (D + 1)], BF16, tag="woe0s")
        nc.sync.dma_start(woe0s, woe0)
        woe1s = cpool.tile([D1 + 1, 2 * (D + 1)], BF16, tag="woe1s")
        nc.sync.dma_start(woe1s, woe1)
        # pre-fault the exp ACT table so the ~2.7us load overlaps input DMAs
        warm = cpool.tile([1, 1], F32, tag="warm")
        nc.scalar.activation(warm, pp[0:1, 0:1],
                             mybir.ActivationFunctionType.Exp)
        # HAM warm-up: ~4us of back-to-back matmuls at kernel start (during
        # the input DMA wait) so the PE clock ramps 1.2 -> 2.4 GHz before the
        # real work begins instead of ~70us in.
        wu_src = cpool.tile([P, QB], FP8, tag="wusrc")
        nc.vector.memset(wu_src, 0.0)
        # 12 back-to-back MMs span >1.5 free-running HAM windows at the cold
        # clock, so the warm transition fires before the real work begins.
        wu_ps = oaccpool.tile([P, QB], F32, tag="oT0", name="warmup")
        for i in range(12):
            nc.tensor.matmul(wu_ps, wu_src[:, 0:P], wu_src,
                             start=(i == 0), stop=(i == 11))

        wqs3 = wqs.rearrange("p (o c) -> p o c", o=2)
        xts3 = xts.rearrange("p (o s) -> p o s", o=2)

        # out accumulator [128, 32*192] f32 (tile g lives at cols g*192)
        out_acc = bigpool.tile([P, NT * D], F32, tag="out_acc")

        # per-attention persistent buffers (distinct tags so att1's phase A
        # can be emitted under att0's ACT-bound phase B)
        # v for both attentions interleaved per row tile: [t, att, VBLK] so
        # one projection matmul + ONE cast serves both (t-stride 416 = 16*26
        # keeps the DoubleRow lhsT step legal).
        vall = bigpool.tile([P, NT * 2 * VBLK], FP8, tag="vall", name="vall")
        bufs = []
        for att in range(2):
            qTd = bigpool.tile([96, 2 * S], FP8, tag=f"qTd{att}", name="qTd")
            kTd = bigpool.tile([96, 2 * S], FP8, tag=f"kTd{att}", name="kTd")
            bufs.append((qTd, kTd))

        COPY = mybir.ActivationFunctionType.Copy

        def qk_unit(att, ci, u, scalar_cast=False):
            """One fp8-DR projection matmul: u 0-1 = q halves, 2-3 = k halves
            of chunk ci for `att`. qTd/kTd layout [96, 2, S], e = 96*o + ki.
            scalar_cast routes the PSUM->fp8 cast to the ScalarE (idle in the
            lead-in / B(0,0)) instead of the DVE."""
            qTd, kTd = bufs[att]
            dst = qTd if u < 2 else kTd
            blk = 2 * att + (u // 2)       # [qr, kr, qw, kw, vr, vw] blocks
            half = u % 2
            woff = blk * D + 96 * half
            ps = mmpool.tile([P, QB], F32, tag="mm", name="ps_proj")
            nc.tensor.matmul(ps[:96, :], wqs3[:, :, woff:woff + 96],
                             xts3[:, :, ci * QB:(ci + 1) * QB],
                             start=True, stop=True, perf_mode=DR)
            d = dst[:, half * S + ci * QB:half * S + (ci + 1) * QB]
            if scalar_cast:
                nc.scalar.activation(d, ps[:96, :], COPY)
            else:
                nc.vector.tensor_copy(d, ps[:96, :])

        vall4 = vall.rearrange("p (t a c) -> p t a c", a=2, c=VBLK)

        def v_unit(t, scalar_cast=False):
            """Row tile t of BOTH attentions' v in one fp8-DR matmul
            (moving operand = [Wv_r | Wv_w], N=384) and one strided cast."""
            if t == 0:
                nc.vector.memset(vall4[:, :, :, D:D + 1], 1.0)
            ps = mmpool.tile([P, QB], F32, tag="mm", name="ps_v")
            nc.tensor.matmul(ps[:, :2 * D], xts3[:, :, t * P:(t + 1) * P],
                             wqs3[:, :, 4 * D:6 * D],
                             start=True, stop=True, perf_mode=DR)
            d = vall4[:, t, :, 0:D]
            srcv = ps[:, 0:2 * D].rearrange("p (a c) -> p a c", c=D)
            if scalar_cast:
                nc.scalar.activation(d, srcv, COPY)
            else:
                nc.vector.tensor_copy(d, srcv)

        NPR = NKC // 2
        ostate = {}

        def phaseB_main(att, qb, interleave=None, pre_oT=None):
            """The first HEAD score-pairs + ACTs are emitted before pre_oT
            (the deferred previous epilogue) so the PE has dependency-ready
            work queued ahead of the epilogue matmuls (which wait on DVE
            copies), and each ot(pr-HEAD) rides with sc_act(pr) so score
            matmuls (ACT's feed) are never FIFO-blocked behind oT work. The
            oT accumulators are allocated after pre_oT so the epi's res
            generations in the shared mm slot form a forward WAR chain."""
            qTd, kTd = bufs[att]
            kT3 = kTd.rearrange("p (o s) -> p o s", o=2)
            qT3 = qTd.rearrange("p (o s) -> p o s", o=2)
            ve3 = vall.rearrange("p (t c) -> p t c", c=2 * VBLK)[
                :, :, att * VBLK:(att + 1) * VBLK]
            qs3 = qT3[:, :, qb * QB:(qb + 1) * QB]
            HEAD = 3
            at3s = {}

            def sc_act(pr):
                # two key-chunks' scoresT side by side in one 2-bank tile
                sc = mmpool.tile([P, 2 * QB], F32, tag="mm", name="sc")
                for h in range(2):
                    kc = 2 * pr + h
                    nc.tensor.matmul(sc[:, h * QB:(h + 1) * QB],
                                     kT3[:, :, kc * KC:(kc + 1) * KC],
                                     qs3, start=True, stop=True,
                                     perf_mode=DR)
                at = apool.tile([P, 2 * QB], FP8, tag="at")
                nc.scalar.activation(at, sc, mybir.ActivationFunctionType.Exp,
                                     scale=SCALE)
                at3s[pr] = at.rearrange("p (o n) -> p o n", o=2)

            def ot(pr, oT0, oT1):
                at3 = at3s.pop(pr)
                nc.tensor.matmul(oT0, ve3[:, 2 * pr:2 * pr + 2, 0:D0], at3,
                                 start=(pr == 0), stop=(pr == NPR - 1),
                                 perf_mode=DR)
                nc.tensor.matmul(oT1, ve3[:, 2 * pr:2 * pr + 2, D0:D + 1],
                                 at3, start=(pr == 0), stop=(pr == NPR - 1),
                                 perf_mode=DR)

            for pr in range(HEAD):
                sc_act(pr)
                if interleave is not None:
                    interleave(pr)
            if pre_oT is not None:
                pre_oT()
            oT0 = oaccpool.tile([P, QB], F32, tag="oT0")
            oT1 = oaccpool.tile([D1 + 1, QB], F32, tag="oT1")
            ostate[(att, qb)] = (oT0, oT1)
            for pr in range(HEAD, NPR):
                sc_act(pr)
                ot(pr - HEAD, oT0, oT1)
                if interleave is not None:
                    interleave(pr)
            for pr in range(NPR - HEAD, NPR):
                ot(pr, oT0, oT1)

        def phaseB_epi(att, qb, final=False):
            wo_off = att * (D + 1)
            fc = 1 + att
            oT0, oT1 = ostate.pop((att, qb))
            # per-qt copy slices: group qt's projection matmuls unblock after
            # ~450ns (their own slices) instead of the full 1.3us copy pair
            oT0s = opool.tile([P, QB], BF16, tag="oT0s")
            oT1s = opool.tile([D1 + 1, QB], BF16, tag="oT1s")
            for qt in range(4):
                sl = slice(qt * P, (qt + 1) * P)
                nc.vector.tensor_copy(oT0s[:, sl], oT0[:, sl])
                nc.vector.tensor_copy(oT1s[:, sl], oT1[:, sl])

            if att == 0:
                xt4 = xpool.tile([P, 4 * D], F32, tag="xt")
                for qt in range(4):
                    g = qb * 4 + qt
                    nc.sync.dma_start(xt4[:, qt * D:(qt + 1) * D],
                                      x[g * P:(g + 1) * P, :])
            for qt in range(4):
                g = qb * 4 + qt
                # res rides the mm slot; the quick resS copy frees it so the
                # next block's score matmuls rotate through unimpeded while
                # the normalize chain reads the SBUF copy off-path.
                res = mmpool.tile([P, QB], F32, tag="mm", name="res")
                res = res[:, 0:D + 1]
                nc.tensor.matmul(res, oT0s[:, qt * P:(qt + 1) * P],
                                 woe0s[:, wo_off:wo_off + D + 1],
                                 start=True, stop=False)
                nc.tensor.matmul(res, oT1s[:, qt * P:(qt + 1) * P],
                                 woe1s[:, wo_off:wo_off + D + 1],
                                 start=False, stop=True)
                resS = tpool.tile([P, D + 1], F32, tag="resS")
                nc.vector.tensor_copy(resS, res)
                rec = tpool.tile([P, 1], F32, tag="rec")
                nc.vector.reciprocal(rec, resS[:, D:D + 1])
                recf = tpool.tile([P, 1], F32, tag="recf")
                nc.vector.tensor_scalar(recf, rec, pp[:, fc:fc + 1], None,
                                        op0=MULT)
                acc = out_acc[:, g * D:(g + 1) * D]
                if att == 0:
                    nc.vector.tensor_scalar(acc, xt4[:, qt * D:(qt + 1) * D],
                                            pp[:, 0:1], None, op0=MULT)
                nc.vector.scalar_tensor_tensor(acc, resS[:, 0:D], recf, acc,
                                               op0=MULT, op1=ADD)
                if att == 1 and final:
                    # last block: finish + ship each group as soon as its
                    # normalize lands so the out DMAs overlap the chain
                    nc.vector.memset(acc[:, MEM_READ:MEM_WRITE + 1], 0.0)
                    nc.vector.tensor_copy(acc[:, MEM_READY:MEM_READY + 1],
                                          pp[:, 3:4])
                    nc.sync.dma_start(out[g * P:(g + 1) * P, :], acc)
            if att == 1 and not final:
                a4 = out_acc.rearrange("p (t c) -> p t c", c=D)[
                    :, qb * 4:(qb + 1) * 4, :]
                nc.vector.memset(a4[:, :, MEM_READ:MEM_WRITE + 1], 0.0)
                nc.vector.tensor_copy(a4[:, :, MEM_READY:MEM_READY + 1],
                                      pp[:, 4:8])
                for qt in range(4):
                    g = qb * 4 + qt
                    nc.sync.dma_start(out[g * P:(g + 1) * P, :],
                                      out_acc[:, g * D:(g + 1) * D])

        # driver: A(0,0)+v(0) head feeds B(0,0); remaining k/v stream JIT
        # under B(0,0) (chunk ci ready one pr-pair before its first use);
        # att1 q/k and att0's next q ride under B(0,qb); epilogues deferred
        # one qb so the next qb's score matmuls keep ACT fed.
        KQ = (2, 3, 0, 1)   # k halves first, then q halves

        def ilv0(pr):
            # JIT prep under B(0,0): k chunk pr//2+1 feeds sc_act(2c) at slot
            # 2c; v chunk pr//2 feeds ot(2c) emitted at slot 2c+HEAD (looser
            # deadline thanks to the oT lag, so v chunk 0 rides slots 0-1
            # instead of the pre-loop head).
            ck = pr // 2 + 1
            cv = pr // 2
            if pr % 2 == 0:
                if ck < NQB:
                    qk_unit(0, ck, 2)
                v_unit(4 * cv + 0, scalar_cast=True)
                v_unit(4 * cv + 1)
            else:
                if ck < NQB:
                    qk_unit(0, ck, 3)
                v_unit(4 * cv + 2, scalar_cast=True)
                v_unit(4 * cv + 3)
            if pr in (14, 15):
                qk_unit(0, 1, pr - 14)

        # A-units ride late prs so their DVE casts queue after the epilogue's
        # DVE chain (which runs at the head of each block).
        def ilv_b0(qb):
            def f(pr):
                if pr >= 8 and pr % 2 == 0:
                    qk_unit(1, qb - 1, KQ[(pr - 8) // 2])
                elif pr in (9, 11) and qb + 1 < NQB:
                    qk_unit(0, qb + 1, (pr - 9) // 2)
            return f

        def ilv_a1_last(pr):
            if pr >= 8 and pr % 2 == 0:
                qk_unit(1, NQB - 1, KQ[(pr - 8) // 2])

        def epi_hook(att, qb):
            return lambda: phaseB_epi(att, qb)

        for u in KQ:
            qk_unit(0, 0, u, scalar_cast=True)
        phaseB_main(0, 0, interleave=ilv0)
        for qb in range(1, NQB):
            phaseB_main(0, qb, interleave=ilv_b0(qb),
                        pre_oT=epi_hook(0, qb - 1))
        # A(1) chunk 7 rides under B(1,0)'s first pairs
        phaseB_main(1, 0, interleave=ilv_a1_last,
                    pre_oT=epi_hook(0, NQB - 1))
        for qb in range(1, NQB):
            phaseB_main(1, qb, pre_oT=epi_hook(1, qb - 1))
        phaseB_epi(1, NQB - 1, final=True)


def _prep_core_inputs(x_full, weights):
    """Host-side shard/layout prep. weights: dict of the 8 [192,192] f32."""
    bf = ml_dtypes.bfloat16
    f8 = ml_dtypes.float8_e4m3

    def to_dr(a):  # [192, C] -> DoubleRow layout [96, 2*C], d = 96*o + ki
        c = a.shape[1]
        return np.ascontiguousarray(
            a.reshape(2, 96, c).transpose(1, 0, 2).reshape(96, 2 * c))

    worder = ["Wq_r", "Wk_r", "Wq_w", "Wk_w", "Wv_r", "Wv_w"]
    wcat = np.concatenate([np.ascontiguousarray(weights[n].T) for n in worder],
                          axis=1)  # [192, 6*192]
    wqkvd = to_dr(wcat).astype(f8)
    woe = np.zeros((D + 1, 2 * (D + 1)), np.float32)
    for a, n in enumerate(("Wo_r", "Wo_w")):
        woe[:D, a * (D + 1):a * (D + 1) + D] = weights[n].T
        woe[D, a * (D + 1) + D] = 1.0
    woe = woe.astype(bf)
    in_maps = []
    for c in range(N_CORES):
        xb = np.ascontiguousarray(x_full[c]).astype(np.float32)  # [4096,192]
        xT = np.ascontiguousarray(xb.T)                          # [192,4096]
        rg = float(xb[0, MEM_READ])
        wg = float(xb[0, MEM_WRITE])
        ready = rg + wg
        pvec = np.array([1.0 - rg - wg, rg, wg, ready,
                         ready, ready, ready, ready], np.float32)
        in_maps.append({
            "x": xb,
            "xtd": to_dr(xT).astype(f8),
            "wqkvd": wqkvd,
            "woe0": np.ascontiguousarray(woe[:D0]),
            "woe1": np.ascontiguousarray(woe[D0:]),
            "params": np.tile(pvec, (P, 1)),
        })
    return in_maps


def _run(inputs, **spmd_kwargs):
    if "nc" not in _CACHE:
        _CACHE["nc"] = _build()
    nc = _CACHE["nc"]
    x_full = np.asarray(inputs["x"], np.float32)
    weights = {k: np.asarray(inputs[k], np.float32) for k in
               ("Wq_r", "Wk_r", "Wv_r", "Wo_r", "Wq_w", "Wk_w", "Wv_w", "Wo_w")}
    in_maps = _prep_core_inputs(x_full, weights)
    res = run_bass_kernel_spmd(nc, in_maps, list(range(N_CORES)), **spmd_kwargs)
    out = np.stack([res.results[c]["out"] for c in range(N_CORES)], axis=0)
    return out.astype(np.float32), res


def kernel(**inputs):
    out, _ = _run(inputs)
    return out


def kernel_traced(**inputs):
    """For test.py: also returns BassKernelResults with profile info."""
    return _run(inputs, trace=True)
